# revision 7
# baseline (speedup 1.0000x reference)
"""AttentionGCN Trainium2 kernel: 8-core SPMD, data-parallel over (batch, row-shard).

Reference computation (B=4, N=4096, F_IN=128, F_OUT=64):
    h = x @ W_fc + b_fc                               [B, N, 64]
    e[b,i,j] = tanh(h[j].w1 + h[i].w2 + b_att)
    a = softmax(adj + e, axis=-1)
    out = a @ h                                       [B, N, 64]

Sharding: core c handles batch b = c//2, rows r0 = (c%2)*2048 .. +2048.
Each core holds full h (all 4096 j) and computes 2048 rows of the score
matrix / softmax / output. No cross-core communication.

Key device-side structure per core:
  - h_full = xT.T @ W_full + B_full where W_full = [W | 0 | W@w1 | W@w2]
    giving columns [h(0:64) | ones(64) | s1(65) | s2'(66)]  (s2' = s2+b_att)
  - S1bc[p, j] = s1[j] broadcast over partitions (ones outer-product via PE)
  - per row-tile (128 rows):
      t    = tanh(S1bc + s2'[i])                (ACT, per-partition bias)
      padd = adj_tile + t                       (DVE)
      pT   = transpose 128x128 chunks of padd   (PE -> PSUM, bf16)
      pexpT= exp(pT)                            (ACT, fused PSUM->SBUF copy)
      oT  += h_ext[jt].T @ pexpT[jt]            (PE, h_ext = [h | ones] so
                                                 row 64 of oT = softmax denom)
      out  = (oT.T)[:, :64] * 1/(oT.T)[:, 64]   (PE transpose + DVE recip/scale)
"""

import sys

if "/opt/trn_rl_repo" not in sys.path:
    sys.path.insert(0, "/opt/trn_rl_repo")

import numpy as np
import ml_dtypes

B, N, F_IN, F_OUT = 4, 4096, 128, 64
N_CORES = 8
R = N // 2          # rows per core (2048)
RT = R // 128       # row tiles per core (16)
JT = N // 128       # j tiles (32)
HW = 67             # h_full width: 64 h + 1 ones + s1 + s2'

_compiled = None


def _build(R=R, N=N, num_devices=N_CORES, stage=3):
    import concourse.bass as bass
    import concourse.tile as tile
    from concourse import bacc, mybir

    RT = R // 128
    JT = N // 128

    f32 = mybir.dt.float32
    bf16 = mybir.dt.bfloat16
    AF = mybir.ActivationFunctionType
    ALU = mybir.AluOpType

    nc = bacc.Bacc("TRN2", target_bir_lowering=False, debug=False,
                   num_devices=num_devices)

    adj_d = nc.dram_tensor("adj_s", [R, N], f32, kind="ExternalInput").ap()
    xT_d = nc.dram_tensor("xT", [F_IN, N], f32, kind="ExternalInput").ap()
    xTs_d = nc.dram_tensor("xTs", [F_IN, R], f32, kind="ExternalInput").ap()
    w_d = nc.dram_tensor("w_mat", [F_IN, F_OUT], f32, kind="ExternalInput").ap()
    w1bc_d = nc.dram_tensor("w1bc", [F_IN, F_OUT], f32, kind="ExternalInput").ap()
    w2bc_d = nc.dram_tensor("w2bc", [F_IN, F_OUT], f32, kind="ExternalInput").ap()
    bfc_d = nc.dram_tensor("bfc", [1, F_OUT], f32, kind="ExternalInput").ap()
    w1r_d = nc.dram_tensor("w1r", [1, F_OUT], f32, kind="ExternalInput").ap()
    w2r_d = nc.dram_tensor("w2r", [1, F_OUT], f32, kind="ExternalInput").ap()
    batt_d = nc.dram_tensor("batt", [1, 1], f32, kind="ExternalInput").ap()
    idbf_d = nc.dram_tensor("id_bf", [128, 128], bf16, kind="ExternalInput").ap()
    idf_d = nc.dram_tensor("id_f32", [128, 128], f32, kind="ExternalInput").ap()
    onesf_d = nc.dram_tensor("ones_f", [1, 128], f32, kind="ExternalInput").ap()
    onesb_d = nc.dram_tensor("ones_b", [1, 128], bf16, kind="ExternalInput").ap()
    out_d = nc.dram_tensor("out", [R, F_OUT], f32, kind="ExternalOutput").ap()

    with tile.TileContext(nc) as tc:
        with (
            tc.tile_pool(name="const", bufs=1) as cpool,
            tc.tile_pool(name="adj", bufs=3) as adj_pool,
            tc.tile_pool(name="tt", bufs=2) as t_pool,
            tc.tile_pool(name="padd", bufs=2) as padd_pool,
            tc.tile_pool(name="pex", bufs=3) as pex_pool,
            tc.tile_pool(name="small", bufs=2) as small_pool,
            tc.tile_pool(name="outp", bufs=2) as out_pool,
            tc.tile_pool(name="ptr", bufs=2, space="PSUM") as ptr_pool,
            tc.tile_pool(name="po", bufs=2, space="PSUM") as po_pool,
            tc.tile_pool(name="pout", bufs=2, space="PSUM") as pout_pool,
        ):
            # ---------------- setup: constants in ----------------
            id_bf = cpool.tile([128, 128], bf16, tag="id_bf")
            nc.sync.dma_start(id_bf[:], idbf_d[:])
            id_f = cpool.tile([128, 128], f32, tag="id_f")
            nc.sync.dma_start(id_f[:], idf_d[:])
            ones_f = cpool.tile([1, 128], f32, tag="ones_f")
            nc.sync.dma_start(ones_f[:], onesf_d[:])
            ones_b = cpool.tile([1, 128], bf16, tag="ones_b")
            nc.sync.dma_start(ones_b[:], onesb_d[:])

            w1bc = cpool.tile([F_IN, F_OUT], f32, tag="w1bc")
            nc.sync.dma_start(w1bc[:], w1bc_d[:])
            w2bc = cpool.tile([F_IN, F_OUT], f32, tag="w2bc")
            nc.sync.dma_start(w2bc[:], w2bc_d[:])
            w1r = cpool.tile([1, F_OUT], f32, tag="w1r")
            nc.sync.dma_start(w1r[:], w1r_d[:])
            w2r = cpool.tile([1, F_OUT], f32, tag="w2r")
            nc.sync.dma_start(w2r[:], w2r_d[:])
            batt = cpool.tile([1, 1], f32, tag="batt")
            nc.sync.dma_start(batt[:], batt_d[:])

            xT = cpool.tile([F_IN, N], f32, tag="xT")
            nc.sync.dma_start(xT[:], xT_d[:])
            xTs = cpool.tile([F_IN, R], f32, tag="xTs")
            nc.sync.dma_start(xTs[:], xTs_d[:])

            # ---------------- W_full = [W | 0 | W@w1 | W@w2] ----------------
            w_full = cpool.tile([F_IN, HW], f32, tag="w_full")
            nc.sync.dma_start(w_full[:, 0:F_OUT], w_d[:])
            nc.vector.memset(w_full[:, F_OUT:F_OUT + 1], 0.0)
            dummy = cpool.tile([F_IN, F_OUT], f32, tag="dummy")
            nc.vector.tensor_mul(dummy[:], w_full[:, 0:F_OUT], w1bc[:])
            nc.vector.tensor_reduce(w_full[:, 65:66], dummy[:],
                                    mybir.AxisListType.X, ALU.add)
            nc.vector.tensor_mul(dummy[:], w_full[:, 0:F_OUT], w2bc[:])
            nc.vector.tensor_reduce(w_full[:, 66:67], dummy[:],
                                    mybir.AxisListType.X, ALU.add)

            # ---------------- bvec = [b_fc | 1 | b.w1 | b.w2+b_att] ----------
            bvec = cpool.tile([1, HW], f32, tag="bvec")
            nc.sync.dma_start(bvec[0:1, 0:F_OUT], bfc_d[:])
            nc.vector.memset(bvec[0:1, F_OUT:F_OUT + 1], 1.0)
            dummy1 = cpool.tile([1, F_OUT], f32, tag="dummy1")
            nc.vector.tensor_mul(dummy1[:], bvec[0:1, 0:F_OUT], w1r[:])
            nc.vector.tensor_reduce(bvec[0:1, 65:66], dummy1[:],
                                    mybir.AxisListType.X, ALU.add)
            nc.vector.tensor_mul(dummy1[:], bvec[0:1, 0:F_OUT], w2r[:])
            nc.vector.tensor_reduce(bvec[0:1, 66:67], dummy1[:],
                                    mybir.AxisListType.X, ALU.add)
            nc.vector.tensor_add(bvec[0:1, 66:67], bvec[0:1, 66:67],
                                 batt[0:1, 0:1])

            # broadcast bvec down partitions: B_sb = ones_f.T @ bvec
            psum_b = pout_pool.tile([128, HW], f32, tag="pout")
            nc.tensor.matmul(psum_b[:], ones_f[:], bvec[:])
            b_sb = cpool.tile([128, HW], f32, tag="b_sb")
            nc.vector.tensor_copy(b_sb[:], psum_b[:])

            # ---------------- h_full over all 32 n-tiles ----------------
            # h_all 2d layout: [128, 32*67], tile nt at cols nt*67 .. +67
            h_all = cpool.tile([128, JT * HW], bf16, tag="h_all")
            s1_all = cpool.tile([128, JT], bf16, tag="s1_all")
            for nt in range(JT):
                psum_hs = pout_pool.tile([128, HW], f32, tag="pout")
                nc.tensor.matmul(psum_hs[:], xT[:, nt * 128:(nt + 1) * 128],
                                 w_full[:])
                nc.vector.tensor_add(h_all[:, nt * HW:(nt + 1) * HW],
                                     psum_hs[:], b_sb[:])
                nc.vector.tensor_copy(s1_all[:, nt:nt + 1],
                                      h_all[:, nt * HW + 65:nt * HW + 66])

            # s2' for this core's shard rows (per row-tile per-partition bias)
            s2_all = cpool.tile([128, RT], f32, tag="s2_all")
            for rt in range(RT):
                psum_s2 = po_pool.tile([128, 1], f32, tag="psum_o")
                nc.tensor.matmul(psum_s2[:], xTs[:, rt * 128:(rt + 1) * 128],
                                 w_full[:, 66:67])
                nc.vector.tensor_add(s2_all[:, rt:rt + 1], psum_s2[:],
                                     b_sb[0:128, 66:67])

            # ---------------- S1bc: s1 broadcast over partitions ------------
            # transpose s1_all [128, 32] -> [32, 128], flatten to a row via
            # sbuf->sbuf DMA (iteration order = partition-major = j order),
            # then ones outer-product chunks of 512.
            psum_s1t = po_pool.tile([JT, 128], bf16, tag="psum_o")
            nc.tensor.transpose(psum_s1t[:], s1_all[:], id_bf[:])
            s1t = cpool.tile([JT, 128], bf16, tag="s1t")
            nc.vector.tensor_copy(s1t[:], psum_s1t[:])
            s1row = cpool.tile([1, N], bf16, tag="s1row")
            nc.sync.dma_start(s1row[:], s1t[:])
            s1bc = cpool.tile([128, N], bf16, tag="s1bc")
            for c in range(N // 512):
                psum_s1bc = ptr_pool.tile([128, 512], f32, tag="ptr")
                nc.tensor.matmul(psum_s1bc[:], ones_b[:],
                                 s1row[0:1, c * 512:(c + 1) * 512])
                nc.scalar.copy(s1bc[:, c * 512:(c + 1) * 512], psum_s1bc[:])

            if stage == 1:
                # debug: out rows <- h_all tile cols + s1bc (forces deps)
                for rt in range(RT):
                    tmp = out_pool.tile([128, F_OUT], f32, tag="out_t")
                    nc.vector.tensor_add(tmp[:], s1bc[:, 0:F_OUT],
                                         h_all[:, rt * HW:rt * HW + F_OUT])
                    nc.vector.tensor_scalar_add(tmp[:], tmp[:],
                                                s2_all[:, rt:rt + 1])
                    nc.sync.dma_start(out_d[rt * 128:(rt + 1) * 128, :], tmp[:])

            # ---------------- main loop over row tiles ----------------
            for rt in range(RT if stage >= 2 else 0):
                adj_t = adj_pool.tile([128, N], f32, tag="adj_t")
                nc.sync.dma_start(adj_t[:], adj_d[rt * 128:(rt + 1) * 128, :])

                t_t = t_pool.tile([128, N], bf16, tag="t_t")
                nc.scalar.activation(t_t[:], s1bc[:], AF.Tanh,
                                     bias=s2_all[:, rt:rt + 1], scale=1.0)

                padd = padd_pool.tile([128, N], bf16, tag="padd")
                nc.vector.tensor_add(padd[:], adj_t[:], t_t[:])

                if stage == 2:
                    tmp = out_pool.tile([128, F_OUT], f32, tag="out_t")
                    nc.vector.tensor_copy(tmp[:], padd[:, 0:F_OUT])
                    nc.sync.dma_start(out_d[rt * 128:(rt + 1) * 128, :], tmp[:])
                    continue

                psum_o = po_pool.tile([65, 128], f32, tag="psum_o")
                G = min(16, JT)
                for half in range(JT // G):
                    ptr = ptr_pool.tile([128, G * 128], bf16, tag="ptr")
                    for k in range(G):
                        jt = half * G + k
                        nc.tensor.transpose(
                            ptr[:, k * 128:(k + 1) * 128],
                            padd[:, jt * 128:(jt + 1) * 128], id_bf[:])
                    pex = pex_pool.tile([128, G * 128], bf16, tag="pex")
                    nc.scalar.activation(pex[:], ptr[:], AF.Exp)
                    for k in range(G):
                        jt = half * G + k
                        nc.tensor.matmul(
                            psum_o[:], h_all[:, jt * HW:jt * HW + 65],
                            pex[:, k * 128:(k + 1) * 128],
                            start=(jt == 0), stop=(jt == JT - 1))

                oT = small_pool.tile([65, 128], f32, tag="oT")
                nc.vector.tensor_copy(oT[:], psum_o[:])
                pout = pout_pool.tile([128, 65], f32, tag="pout")
                nc.tensor.transpose(pout[:], oT[:], id_f[0:65, 0:65])
                rec = small_pool.tile([128, 1], f32, tag="rec")
                nc.vector.reciprocal(rec[:], pout[:, 64:65])
                out_t = out_pool.tile([128, F_OUT], f32, tag="out_t")
                nc.vector.tensor_scalar_mul(out_t[:], pout[:, 0:F_OUT], rec[:])
                nc.sync.dma_start(out_d[rt * 128:(rt + 1) * 128, :], out_t[:])

    nc.compile()
    return nc


def _get_compiled():
    global _compiled
    if _compiled is None:
        _compiled = _build()
    return _compiled


def _make_in_maps(x, adj, W_fc, b_fc, w_att, b_att):
    x = np.asarray(x, dtype=np.float32)
    adj = np.asarray(adj, dtype=np.float32)
    W_fc = np.asarray(W_fc, dtype=np.float32)
    b_fc = np.asarray(b_fc, dtype=np.float32)
    w_att = np.asarray(w_att, dtype=np.float32)
    b_att = np.asarray(b_att, dtype=np.float32)

    w1 = w_att[:F_OUT]
    w2 = w_att[F_OUT:]
    shared = {
        "w_mat": np.ascontiguousarray(W_fc),
        "w1bc": np.ascontiguousarray(np.broadcast_to(w1, (F_IN, F_OUT))),
        "w2bc": np.ascontiguousarray(np.broadcast_to(w2, (F_IN, F_OUT))),
        "bfc": b_fc.reshape(1, F_OUT).copy(),
        "w1r": w1.reshape(1, F_OUT).copy(),
        "w2r": w2.reshape(1, F_OUT).copy(),
        "batt": np.full((1, 1), float(b_att), dtype=np.float32),
        "id_bf": np.eye(128, dtype=ml_dtypes.bfloat16),
        "id_f32": np.eye(128, dtype=np.float32),
        "ones_f": np.ones((1, 128), dtype=np.float32),
        "ones_b": np.ones((1, 128), dtype=ml_dtypes.bfloat16),
    }
    in_maps = []
    for c in range(N_CORES):
        b = c // 2
        r0 = (c % 2) * R
        xT = np.ascontiguousarray(x[b].T)
        m = dict(shared)
        m["adj_s"] = np.ascontiguousarray(adj[b, r0:r0 + R, :])
        m["xT"] = xT
        m["xTs"] = np.ascontiguousarray(xT[:, r0:r0 + R])
        in_maps.append(m)
    return in_maps


def run(x, adj, W_fc, b_fc, w_att, b_att, trace=False):
    from concourse import bass_utils

    nc = _get_compiled()
    in_maps = _make_in_maps(x, adj, W_fc, b_fc, w_att, b_att)
    res = bass_utils.run_bass_kernel_spmd(
        nc, in_maps, core_ids=list(range(N_CORES)), trace=trace)
    out = np.empty((B, N, F_OUT), dtype=np.float32)
    for c in range(N_CORES):
        b = c // 2
        r0 = (c % 2) * R
        out[b, r0:r0 + R, :] = res.results[c]["out"]
    return out, res


def kernel(x, adj, W_fc, b_fc, w_att, b_att):
    out, _ = run(x, adj, W_fc, b_fc, w_att, b_att, trace=False)
    return out


# revision 9
# speedup vs baseline: 1.0407x; 1.0407x over previous
"""AttentionGCN Trainium2 kernel: 8-core SPMD, data-parallel over (batch, row-shard).

Reference computation (B=4, N=4096, F_IN=128, F_OUT=64):
    h = x @ W_fc + b_fc                               [B, N, 64]
    e[b,i,j] = tanh(h[j].w1 + h[i].w2 + b_att)
    a = softmax(adj + e, axis=-1)
    out = a @ h                                       [B, N, 64]

Sharding: core c handles batch b = c//2, rows r0 = (c%2)*2048 .. +2048.
Each core holds full h (all 4096 j) and computes 2048 rows of the score
matrix / softmax / output. No cross-core communication.

Key device-side structure per core:
  - h_full = xT.T @ W_full + B_full where W_full = [W | 0 | W@w1 | W@w2]
    giving columns [h(0:64) | ones(64) | s1(65) | s2'(66)]  (s2' = s2+b_att)
  - S1bc[p, j] = s1[j] broadcast over partitions (ones outer-product via PE)
  - per row-tile (128 rows):
      t    = tanh(S1bc + s2'[i])                (ACT, per-partition bias)
      padd = adj_tile + t                       (DVE)
      pT   = transpose 128x128 chunks of padd   (PE -> PSUM, bf16)
      pexpT= exp(pT)                            (ACT, fused PSUM->SBUF copy)
      oT  += h_ext[jt].T @ pexpT[jt]            (PE, h_ext = [h | ones] so
                                                 row 64 of oT = softmax denom)
      out  = (oT.T)[:, :64] * 1/(oT.T)[:, 64]   (PE transpose + DVE recip/scale)
"""

import sys

if "/opt/trn_rl_repo" not in sys.path:
    sys.path.insert(0, "/opt/trn_rl_repo")

import numpy as np
import ml_dtypes

B, N, F_IN, F_OUT = 4, 4096, 128, 64
N_CORES = 8
R = N // 2          # rows per core (2048)
RT = R // 128       # row tiles per core (16)
JT = N // 128       # j tiles (32)
HW = 67             # h_full width: 64 h + 1 ones + s1 + s2'

_compiled = None


def _build(R=R, N=N, num_devices=N_CORES, stage=3):
    import concourse.bass as bass
    import concourse.tile as tile
    from concourse import bacc, mybir

    RT = R // 128
    JT = N // 128

    f32 = mybir.dt.float32
    bf16 = mybir.dt.bfloat16
    AF = mybir.ActivationFunctionType
    ALU = mybir.AluOpType

    nc = bacc.Bacc("TRN2", target_bir_lowering=False, debug=False,
                   num_devices=num_devices)

    adj_d = nc.dram_tensor("adj_s", [R, N], f32, kind="ExternalInput").ap()
    xT_d = nc.dram_tensor("xT", [F_IN, N], f32, kind="ExternalInput").ap()
    xTs_d = nc.dram_tensor("xTs", [F_IN, R], f32, kind="ExternalInput").ap()
    w_d = nc.dram_tensor("w_mat", [F_IN, F_OUT], f32, kind="ExternalInput").ap()
    w1bc_d = nc.dram_tensor("w1bc", [F_IN, F_OUT], f32, kind="ExternalInput").ap()
    w2bc_d = nc.dram_tensor("w2bc", [F_IN, F_OUT], f32, kind="ExternalInput").ap()
    bfc_d = nc.dram_tensor("bfc", [1, F_OUT], f32, kind="ExternalInput").ap()
    w1r_d = nc.dram_tensor("w1r", [1, F_OUT], f32, kind="ExternalInput").ap()
    w2r_d = nc.dram_tensor("w2r", [1, F_OUT], f32, kind="ExternalInput").ap()
    batt_d = nc.dram_tensor("batt", [1, 1], f32, kind="ExternalInput").ap()
    idbf_d = nc.dram_tensor("id_bf", [128, 128], bf16, kind="ExternalInput").ap()
    idf_d = nc.dram_tensor("id_f32", [128, 128], f32, kind="ExternalInput").ap()
    onesf_d = nc.dram_tensor("ones_f", [1, 128], f32, kind="ExternalInput").ap()
    onesb_d = nc.dram_tensor("ones_b", [1, 128], bf16, kind="ExternalInput").ap()
    out_d = nc.dram_tensor("out", [R, F_OUT], f32, kind="ExternalOutput").ap()

    with tile.TileContext(nc) as tc:
        with (
            tc.tile_pool(name="const", bufs=1) as cpool,
            tc.tile_pool(name="adj", bufs=4) as adj_pool,
            tc.tile_pool(name="tt", bufs=2) as t_pool,
            tc.tile_pool(name="padd", bufs=6) as padd_pool,
            tc.tile_pool(name="pex", bufs=3) as pex_pool,
            tc.tile_pool(name="small", bufs=2) as small_pool,
            tc.tile_pool(name="outp", bufs=2) as out_pool,
            tc.tile_pool(name="ptr", bufs=2, space="PSUM") as ptr_pool,
            tc.tile_pool(name="po", bufs=2, space="PSUM") as po_pool,
            tc.tile_pool(name="pout", bufs=2, space="PSUM") as pout_pool,
        ):
            # ---------------- setup: constants in ----------------
            id_bf = cpool.tile([128, 128], bf16, tag="id_bf")
            nc.sync.dma_start(id_bf[:], idbf_d[:])
            id_f = cpool.tile([128, 128], f32, tag="id_f")
            nc.sync.dma_start(id_f[:], idf_d[:])
            ones_f = cpool.tile([1, 128], f32, tag="ones_f")
            nc.sync.dma_start(ones_f[:], onesf_d[:])
            ones_b = cpool.tile([1, 128], bf16, tag="ones_b")
            nc.sync.dma_start(ones_b[:], onesb_d[:])

            w1bc = cpool.tile([F_IN, F_OUT], f32, tag="w1bc")
            nc.sync.dma_start(w1bc[:], w1bc_d[:])
            w2bc = cpool.tile([F_IN, F_OUT], f32, tag="w2bc")
            nc.sync.dma_start(w2bc[:], w2bc_d[:])
            w1r = cpool.tile([1, F_OUT], f32, tag="w1r")
            nc.sync.dma_start(w1r[:], w1r_d[:])
            w2r = cpool.tile([1, F_OUT], f32, tag="w2r")
            nc.sync.dma_start(w2r[:], w2r_d[:])
            batt = cpool.tile([1, 1], f32, tag="batt")
            nc.sync.dma_start(batt[:], batt_d[:])

            xT = cpool.tile([F_IN, N], f32, tag="xT")
            nc.sync.dma_start(xT[:], xT_d[:])
            xTs = cpool.tile([F_IN, R], f32, tag="xTs")
            nc.sync.dma_start(xTs[:], xTs_d[:])

            # ---------------- W_full = [W | 0 | W@w1 | W@w2] ----------------
            w_full = cpool.tile([F_IN, HW], f32, tag="w_full")
            nc.sync.dma_start(w_full[:, 0:F_OUT], w_d[:])
            nc.vector.memset(w_full[:, F_OUT:F_OUT + 1], 0.0)
            dummy = cpool.tile([F_IN, F_OUT], f32, tag="dummy")
            nc.vector.tensor_mul(dummy[:], w_full[:, 0:F_OUT], w1bc[:])
            nc.vector.tensor_reduce(w_full[:, 65:66], dummy[:],
                                    mybir.AxisListType.X, ALU.add)
            nc.vector.tensor_mul(dummy[:], w_full[:, 0:F_OUT], w2bc[:])
            nc.vector.tensor_reduce(w_full[:, 66:67], dummy[:],
                                    mybir.AxisListType.X, ALU.add)

            # ---------------- bvec = [b_fc | 1 | b.w1 | b.w2+b_att] ----------
            bvec = cpool.tile([1, HW], f32, tag="bvec")
            nc.sync.dma_start(bvec[0:1, 0:F_OUT], bfc_d[:])
            nc.vector.memset(bvec[0:1, F_OUT:F_OUT + 1], 1.0)
            dummy1 = cpool.tile([1, F_OUT], f32, tag="dummy1")
            nc.vector.tensor_mul(dummy1[:], bvec[0:1, 0:F_OUT], w1r[:])
            nc.vector.tensor_reduce(bvec[0:1, 65:66], dummy1[:],
                                    mybir.AxisListType.X, ALU.add)
            nc.vector.tensor_mul(dummy1[:], bvec[0:1, 0:F_OUT], w2r[:])
            nc.vector.tensor_reduce(bvec[0:1, 66:67], dummy1[:],
                                    mybir.AxisListType.X, ALU.add)
            nc.vector.tensor_add(bvec[0:1, 66:67], bvec[0:1, 66:67],
                                 batt[0:1, 0:1])

            # broadcast bvec down partitions: B_sb = ones_f.T @ bvec
            psum_b = pout_pool.tile([128, HW], f32, tag="pout")
            nc.tensor.matmul(psum_b[:], ones_f[:], bvec[:])
            b_sb = cpool.tile([128, HW], f32, tag="b_sb")
            nc.vector.tensor_copy(b_sb[:], psum_b[:])

            # ---------------- h_full over all 32 n-tiles ----------------
            # h_all 2d layout: [128, 32*67], tile nt at cols nt*67 .. +67
            h_all = cpool.tile([128, JT * HW], bf16, tag="h_all")
            s1_all = cpool.tile([128, JT], bf16, tag="s1_all")
            for nt in range(JT):
                psum_hs = pout_pool.tile([128, HW], f32, tag="pout")
                nc.tensor.matmul(psum_hs[:], xT[:, nt * 128:(nt + 1) * 128],
                                 w_full[:])
                nc.vector.tensor_add(h_all[:, nt * HW:(nt + 1) * HW],
                                     psum_hs[:], b_sb[:])
                nc.vector.tensor_copy(s1_all[:, nt:nt + 1],
                                      h_all[:, nt * HW + 65:nt * HW + 66])

            # s2' for this core's shard rows (per row-tile per-partition bias)
            s2_all = cpool.tile([128, RT], f32, tag="s2_all")
            for rt in range(RT):
                psum_s2 = po_pool.tile([128, 1], f32, tag="psum_o")
                nc.tensor.matmul(psum_s2[:], xTs[:, rt * 128:(rt + 1) * 128],
                                 w_full[:, 66:67])
                nc.vector.tensor_add(s2_all[:, rt:rt + 1], psum_s2[:],
                                     b_sb[0:128, 66:67])

            # ---------------- S1bc: s1 broadcast over partitions ------------
            # transpose s1_all [128, 32] -> [32, 128], flatten to a row via
            # sbuf->sbuf DMA (iteration order = partition-major = j order),
            # then ones outer-product chunks of 512.
            psum_s1t = po_pool.tile([JT, 128], bf16, tag="psum_o")
            nc.tensor.transpose(psum_s1t[:], s1_all[:], id_bf[:])
            s1t = cpool.tile([JT, 128], bf16, tag="s1t")
            nc.vector.tensor_copy(s1t[:], psum_s1t[:])
            s1row = cpool.tile([1, N], bf16, tag="s1row")
            nc.sync.dma_start(s1row[:], s1t[:])
            s1bc = cpool.tile([128, N], bf16, tag="s1bc")
            for c in range(N // 512):
                psum_s1bc = ptr_pool.tile([128, 512], f32, tag="ptr")
                nc.tensor.matmul(psum_s1bc[:], ones_b[:],
                                 s1row[0:1, c * 512:(c + 1) * 512])
                nc.scalar.copy(s1bc[:, c * 512:(c + 1) * 512], psum_s1bc[:])

            if stage == 1:
                # debug: out rows <- h_all tile cols + s1bc (forces deps)
                for rt in range(RT):
                    tmp = out_pool.tile([128, F_OUT], f32, tag="out_t")
                    nc.vector.tensor_add(tmp[:], s1bc[:, 0:F_OUT],
                                         h_all[:, rt * HW:rt * HW + F_OUT])
                    nc.vector.tensor_scalar_add(tmp[:], tmp[:],
                                                s2_all[:, rt:rt + 1])
                    nc.sync.dma_start(out_d[rt * 128:(rt + 1) * 128, :], tmp[:])

            # ---------------- main loop over row-tile groups ----------------
            # RG row tiles per group; per (group, jb) one psum ptr tile holds
            # KJ j-tiles x RG row-tiles of transposed 128x128 chunks so the
            # PV matmul's moving operand spans RG*128 output columns.
            RG = 4 if (RT % 4 == 0 and JT % 4 == 0) else 1
            KJ = 16 // RG if JT % (16 // RG) == 0 else min(16, JT)
            for g in range(RT // RG if stage >= 2 else 0):
                padds = []
                for r in range(RG):
                    rt = g * RG + r
                    adj_t = adj_pool.tile([128, N], f32, tag="adj_t")
                    nc.sync.dma_start(adj_t[:],
                                      adj_d[rt * 128:(rt + 1) * 128, :])
                    t_t = t_pool.tile([128, N], bf16, tag="t_t")
                    nc.scalar.activation(t_t[:], s1bc[:], AF.Tanh,
                                         bias=s2_all[:, rt:rt + 1], scale=1.0)
                    padd = padd_pool.tile([128, N], bf16, tag="padd")
                    nc.vector.tensor_add(padd[:], adj_t[:], t_t[:])
                    padds.append(padd)

                if stage == 2:
                    for r in range(RG):
                        rt = g * RG + r
                        tmp = out_pool.tile([128, F_OUT], f32, tag="out_t")
                        nc.vector.tensor_copy(tmp[:], padds[r][:, 0:F_OUT])
                        nc.sync.dma_start(out_d[rt * 128:(rt + 1) * 128, :],
                                          tmp[:])
                    continue

                psum_o = po_pool.tile([65, RG * 128], f32, tag="psum_o")
                for jb in range(JT // KJ):
                    ptr = ptr_pool.tile([128, KJ * RG * 128], bf16, tag="ptr")
                    for k in range(KJ):
                        jt = jb * KJ + k
                        for r in range(RG):
                            nc.tensor.transpose(
                                ptr[:, (k * RG + r) * 128:(k * RG + r + 1) * 128],
                                padds[r][:, jt * 128:(jt + 1) * 128], id_bf[:])
                    pex = pex_pool.tile([128, KJ * RG * 128], bf16, tag="pex")
                    nc.scalar.activation(pex[:], ptr[:], AF.Exp)
                    for k in range(KJ):
                        jt = jb * KJ + k
                        nc.tensor.matmul(
                            psum_o[:], h_all[:, jt * HW:jt * HW + 65],
                            pex[:, k * RG * 128:(k + 1) * RG * 128],
                            start=(jt == 0), stop=(jt == JT - 1))

                oT = small_pool.tile([65, RG * 128], f32, tag="oT")
                nc.vector.tensor_copy(oT[:], psum_o[:])
                for r in range(RG):
                    rt = g * RG + r
                    pout = pout_pool.tile([128, 65], f32, tag="pout")
                    nc.tensor.transpose(pout[:], oT[:, r * 128:(r + 1) * 128],
                                        id_f[0:65, 0:65])
                    rec = small_pool.tile([128, 1], f32, tag="rec")
                    nc.vector.reciprocal(rec[:], pout[:, 64:65])
                    out_t = out_pool.tile([128, F_OUT], f32, tag="out_t")
                    nc.vector.tensor_scalar_mul(out_t[:], pout[:, 0:F_OUT],
                                                rec[:])
                    nc.sync.dma_start(out_d[rt * 128:(rt + 1) * 128, :],
                                      out_t[:])

    nc.compile()
    return nc


def _get_compiled():
    global _compiled
    if _compiled is None:
        _compiled = _build()
    return _compiled


def _make_in_maps(x, adj, W_fc, b_fc, w_att, b_att):
    x = np.asarray(x, dtype=np.float32)
    adj = np.asarray(adj, dtype=np.float32)
    W_fc = np.asarray(W_fc, dtype=np.float32)
    b_fc = np.asarray(b_fc, dtype=np.float32)
    w_att = np.asarray(w_att, dtype=np.float32)
    b_att = np.asarray(b_att, dtype=np.float32)

    w1 = w_att[:F_OUT]
    w2 = w_att[F_OUT:]
    shared = {
        "w_mat": np.ascontiguousarray(W_fc),
        "w1bc": np.ascontiguousarray(np.broadcast_to(w1, (F_IN, F_OUT))),
        "w2bc": np.ascontiguousarray(np.broadcast_to(w2, (F_IN, F_OUT))),
        "bfc": b_fc.reshape(1, F_OUT).copy(),
        "w1r": w1.reshape(1, F_OUT).copy(),
        "w2r": w2.reshape(1, F_OUT).copy(),
        "batt": np.full((1, 1), float(b_att), dtype=np.float32),
        "id_bf": np.eye(128, dtype=ml_dtypes.bfloat16),
        "id_f32": np.eye(128, dtype=np.float32),
        "ones_f": np.ones((1, 128), dtype=np.float32),
        "ones_b": np.ones((1, 128), dtype=ml_dtypes.bfloat16),
    }
    in_maps = []
    for c in range(N_CORES):
        b = c // 2
        r0 = (c % 2) * R
        xT = np.ascontiguousarray(x[b].T)
        m = dict(shared)
        m["adj_s"] = np.ascontiguousarray(adj[b, r0:r0 + R, :])
        m["xT"] = xT
        m["xTs"] = np.ascontiguousarray(xT[:, r0:r0 + R])
        in_maps.append(m)
    return in_maps


def run(x, adj, W_fc, b_fc, w_att, b_att, trace=False):
    from concourse import bass_utils

    nc = _get_compiled()
    in_maps = _make_in_maps(x, adj, W_fc, b_fc, w_att, b_att)
    res = bass_utils.run_bass_kernel_spmd(
        nc, in_maps, core_ids=list(range(N_CORES)), trace=trace)
    out = np.empty((B, N, F_OUT), dtype=np.float32)
    for c in range(N_CORES):
        b = c // 2
        r0 = (c % 2) * R
        out[b, r0:r0 + R, :] = res.results[c]["out"]
    return out, res


def kernel(x, adj, W_fc, b_fc, w_att, b_att):
    out, _ = run(x, adj, W_fc, b_fc, w_att, b_att, trace=False)
    return out
